# revision 2
# baseline (speedup 1.0000x reference)
"""Trainium2 Bass kernel for BiochemicalDynamics.

Reference computation (f32):
    Ax    = A @ x                                   # [N, DIM]
    s     = R * rowsum(x * Ax)                      # [N, 1]
    out   = F - B*x - s                             # [N, DIM]

Key identity used on-device: the output only needs the per-row scalar
    s_i = R * sum_j A[i,j] * <x_i, x_j> = R * rowsum_j (A ⊙ G)[i,j]
with G = x @ x.T. G tiles are produced on the TensorEngine from xT
(stationary xT[:, rows_i], moving xT[:, cols_j]) — so A is consumed in
its natural row-major layout and never needs a transpose. A single
fused VectorEngine op (tensor_tensor_reduce) multiplies the A chunk by
the G chunk from PSUM and row-reduces it, chaining the per-partition
accumulator across chunks.

Sharding: row-shard A (and x rows) across the 8 cores; every core gets
the full xT (the "all-gather of x" is done host-side by replicating the
2MB input). No cross-core reduction is needed.
"""

import sys

import numpy as np

for _p in ("/opt/trn_rl_repo", "/root/.axon_site/_ro/trn_rl_repo"):
    if _p not in sys.path:
        sys.path.append(_p)

N = 8192
DIM = 64
NCORES = 8
ROWS = N // NCORES  # 1024 rows of A per core

F_CONST = 1.0
B_CONST = 0.1
R_CONST = 0.01

P = 128                  # SBUF partitions
NSTRIPES = ROWS // P     # 8 row-stripes per core
CHUNK = 2048             # columns per fused multiply-reduce (4 PSUM banks)
NCHUNKS = N // CHUNK     # 4
MM_N = 512               # matmul moving free dim (one PSUM bank, f32)
MM_PER_CHUNK = CHUNK // MM_N

_CACHE = {}


def _build_nc():
    import concourse.mybir as mybir
    import concourse.tile as tile
    from concourse import bacc

    f32 = mybir.dt.float32

    nc = bacc.Bacc(
        trn_type="TRN2", target_bir_lowering=False, debug=False, num_devices=NCORES
    )

    a = nc.dram_tensor("a", [ROWS, N], f32, kind="ExternalInput")
    xt = nc.dram_tensor("xt", [DIM, N], f32, kind="ExternalInput")
    xlt = nc.dram_tensor("xlt", [DIM, ROWS], f32, kind="ExternalInput")
    xloc = nc.dram_tensor("xloc", [ROWS, DIM], f32, kind="ExternalInput")
    out = nc.dram_tensor("out", [ROWS, DIM], f32, kind="ExternalOutput")

    mult = mybir.AluOpType.mult
    add = mybir.AluOpType.add

    with tile.TileContext(nc) as tc:
        with (
            tc.tile_pool(name="xpool", bufs=1) as xpool,
            tc.tile_pool(name="apool", bufs=3) as apool,
            tc.tile_pool(name="spool", bufs=2) as spool,
            tc.tile_pool(name="accpool", bufs=2 * NCHUNKS) as accpool,
            tc.tile_pool(name="psum", bufs=2, space="PSUM") as psum_pool,
        ):
            # One-time loads: x^T (rhs for G matmuls) and the local-row
            # slice of x^T (stationary operands).
            xt_sb = xpool.tile([DIM, N], f32)
            nc.sync.dma_start(out=xt_sb[:], in_=xt[:])
            xlt_sb = xpool.tile([DIM, ROWS], f32)
            nc.sync.dma_start(out=xlt_sb[:], in_=xlt[:])

            for s in range(NSTRIPES):
                a_sb = apool.tile([P, N], f32, tag="a")
                nc.sync.dma_start(out=a_sb[:], in_=a[s * P : (s + 1) * P, :])
                xl_sb = spool.tile([P, DIM], f32, tag="xl")
                nc.sync.dma_start(out=xl_sb[:], in_=xloc[s * P : (s + 1) * P, :])

                # acc4[:, c] = sum_j (A_chunk * R) * G_chunk  per chunk c,
                # via the fused DVE scalar_tensor_tensor accumulate output.
                acc4 = accpool.tile([P, NCHUNKS], f32, tag="acc4")
                for c in range(NCHUNKS):
                    g_ps = psum_pool.tile([P, CHUNK], f32, tag="g")
                    for q in range(MM_PER_CHUNK):
                        col = c * CHUNK + q * MM_N
                        nc.tensor.matmul(
                            g_ps[:, q * MM_N : (q + 1) * MM_N],
                            xlt_sb[:, s * P : (s + 1) * P],
                            xt_sb[:, col : col + MM_N],
                            start=True,
                            stop=True,
                        )
                    dummy = accpool.tile([P, 1], f32, tag="dummy")
                    nc.vector.scalar_tensor_tensor(
                        dummy.broadcast_to((P, CHUNK)),
                        a_sb[:, c * CHUNK : (c + 1) * CHUNK],
                        R_CONST,
                        g_ps[:],
                        op0=mult,
                        op1=mult,
                        accum_out=acc4[:, c : c + 1],
                    )

                # s = rowsum(acc4) ; v = F - s ; out = (x * -B) + v
                s_acc = accpool.tile([P, 1], f32, tag="s")
                nc.vector.tensor_reduce(s_acc, acc4[:], mybir.AxisListType.X, add)
                v = accpool.tile([P, 1], f32, tag="v")
                nc.scalar.activation(
                    v, s_acc, mybir.ActivationFunctionType.Copy,
                    bias=F_CONST, scale=-1.0,
                )
                o_sb = spool.tile([P, DIM], f32, tag="o")
                nc.vector.tensor_scalar(
                    out=o_sb, in0=xl_sb, scalar1=-B_CONST, scalar2=v,
                    op0=mult, op1=add,
                )
                nc.sync.dma_start(out=out[s * P : (s + 1) * P, :], in_=o_sb[:])

    nc.finalize()
    return nc


def _get_nc():
    if "nc" not in _CACHE:
        _CACHE["nc"] = _build_nc()
    return _CACHE["nc"]


def _make_in_maps(x, A):
    x = np.ascontiguousarray(np.asarray(x, dtype=np.float32))
    A = np.ascontiguousarray(np.asarray(A, dtype=np.float32))
    xt = np.ascontiguousarray(x.T)
    in_maps = []
    for c in range(NCORES):
        rows = slice(c * ROWS, (c + 1) * ROWS)
        in_maps.append(
            {
                "a": np.ascontiguousarray(A[rows]),
                "xt": xt,
                "xlt": np.ascontiguousarray(xt[:, rows]),
                "xloc": np.ascontiguousarray(x[rows]),
            }
        )
    return in_maps


def run_sharded(x, A, trace=False, **kwargs):
    """Run the SPMD bass kernel; returns (full_output, BassKernelResults)."""
    from concourse.bass_utils import run_bass_kernel_spmd

    nc = _get_nc()
    res = run_bass_kernel_spmd(
        nc, _make_in_maps(x, A), core_ids=list(range(NCORES)), trace=trace, **kwargs
    )
    full = np.concatenate([res.results[c]["out"] for c in range(NCORES)], axis=0)
    return full.astype(np.float32, copy=False), res


def kernel(t, x, A):
    out, _ = run_sharded(x, A)
    return out


# revision 6
# speedup vs baseline: 1.2729x; 1.2729x over previous
"""Trainium2 Bass kernel for BiochemicalDynamics.

Reference computation (f32):
    Ax    = A @ x                                   # [N, DIM]
    s     = R * rowsum(x * Ax)                      # [N, 1]
    out   = F - B*x - s                             # [N, DIM]

Key identity used on-device: the output only needs the per-row scalar
    s_i = R * sum_j A[i,j] * <x_i, x_j> = R * rowsum_j (A ⊙ G)[i,j]
with G = x @ x.T. G tiles are produced on the TensorEngine from xT
(stationary xT[:, rows_i], moving xT[:, cols_j]) — so A is consumed in
its natural row-major layout and never needs a transpose. A single
fused VectorEngine op (tensor_tensor_reduce) multiplies the A chunk by
the G chunk from PSUM and row-reduces it, chaining the per-partition
accumulator across chunks.

Sharding: row-shard A (and x rows) across the 8 cores; every core gets
the full xT (the "all-gather of x" is done host-side by replicating the
2MB input). No cross-core reduction is needed.
"""

import sys

import numpy as np

for _p in ("/opt/trn_rl_repo", "/root/.axon_site/_ro/trn_rl_repo"):
    if _p not in sys.path:
        sys.path.append(_p)

N = 8192
DIM = 64
NCORES = 8
ROWS = N // NCORES  # 1024 rows of A per core

F_CONST = 1.0
B_CONST = 0.1
R_CONST = 0.01

P = 128                  # SBUF partitions
NSTRIPES = ROWS // P     # 8 row-stripes per core
CHUNK = 2048             # columns per fused multiply-reduce (4 PSUM banks)
NCHUNKS = N // CHUNK     # 4
MM_N = 512               # matmul moving free dim (one PSUM bank, f32)
MM_PER_CHUNK = CHUNK // MM_N

_CACHE = {}


def _build_nc():
    import concourse.mybir as mybir
    import concourse.tile as tile
    from concourse import bacc

    f32 = mybir.dt.float32
    bf16 = mybir.dt.bfloat16

    nc = bacc.Bacc(
        trn_type="TRN2", target_bir_lowering=False, debug=False, num_devices=NCORES
    )

    a = nc.dram_tensor("a", [ROWS, N], f32, kind="ExternalInput")
    # x^T split into bf16 (hi, lo) pairs: x = hi + lo to ~2^-17. The G
    # matmuls run in bf16 (4-5x faster than fp32 on PE) with f32 PSUM
    # accumulation: G = hi·hiT + hi·loT + lo·hiT (lo·loT ~2^-18, dropped).
    xt_hi = nc.dram_tensor("xt_hi", [DIM, N], bf16, kind="ExternalInput")
    xt_lo = nc.dram_tensor("xt_lo", [DIM, N], bf16, kind="ExternalInput")
    xlt_hi = nc.dram_tensor("xlt_hi", [DIM, ROWS], bf16, kind="ExternalInput")
    xlt_lo = nc.dram_tensor("xlt_lo", [DIM, ROWS], bf16, kind="ExternalInput")
    xloc = nc.dram_tensor("xloc", [ROWS, DIM], f32, kind="ExternalInput")
    out = nc.dram_tensor("out", [ROWS, DIM], f32, kind="ExternalOutput")

    mult = mybir.AluOpType.mult
    add = mybir.AluOpType.add

    with tile.TileContext(nc) as tc:
        with (
            tc.tile_pool(name="xpool", bufs=1) as xpool,
            tc.tile_pool(name="apool", bufs=3) as apool,
            tc.tile_pool(name="spool", bufs=2) as spool,
            tc.tile_pool(name="accpool", bufs=2 * NCHUNKS) as accpool,
            tc.tile_pool(name="psum", bufs=2, space="PSUM") as psum_pool,
        ):
            # One-time loads: x^T hi/lo (rhs for G matmuls) and the
            # local-row slices of x^T (stationary operands).
            xt_hi_sb = xpool.tile([DIM, N], bf16)
            nc.sync.dma_start(out=xt_hi_sb[:], in_=xt_hi[:])
            xt_lo_sb = xpool.tile([DIM, N], bf16)
            nc.sync.dma_start(out=xt_lo_sb[:], in_=xt_lo[:])
            xlt_hi_sb = xpool.tile([DIM, ROWS], bf16)
            nc.sync.dma_start(out=xlt_hi_sb[:], in_=xlt_hi[:])
            xlt_lo_sb = xpool.tile([DIM, ROWS], bf16)
            nc.sync.dma_start(out=xlt_lo_sb[:], in_=xlt_lo[:])

            for s in range(NSTRIPES):
                a_sb = apool.tile([P, N], f32, tag="a")
                nc.sync.dma_start(out=a_sb[:], in_=a[s * P : (s + 1) * P, :])
                xl_sb = spool.tile([P, DIM], f32, tag="xl")
                nc.sync.dma_start(out=xl_sb[:], in_=xloc[s * P : (s + 1) * P, :])

                # acc4[:, c] = sum_j (A_chunk * R) * G_chunk  per chunk c,
                # via the fused DVE scalar_tensor_tensor accumulate output.
                acc4 = accpool.tile([P, NCHUNKS], f32, tag="acc4")
                for c in range(NCHUNKS):
                    g_ps = psum_pool.tile([P, CHUNK], f32, tag="g")
                    for q in range(MM_PER_CHUNK):
                        col = c * CHUNK + q * MM_N
                        g_slice = g_ps[:, q * MM_N : (q + 1) * MM_N]
                        lt_hi = xlt_hi_sb[:, s * P : (s + 1) * P]
                        lt_lo = xlt_lo_sb[:, s * P : (s + 1) * P]
                        rt_hi = xt_hi_sb[:, col : col + MM_N]
                        rt_lo = xt_lo_sb[:, col : col + MM_N]
                        nc.tensor.matmul(g_slice, lt_hi, rt_hi, start=True, stop=False)
                        nc.tensor.matmul(g_slice, lt_hi, rt_lo, start=False, stop=False)
                        nc.tensor.matmul(g_slice, lt_lo, rt_hi, start=False, stop=True)
                    dummy = accpool.tile([P, 1], f32, tag="dummy")
                    nc.vector.scalar_tensor_tensor(
                        dummy.broadcast_to((P, CHUNK)),
                        a_sb[:, c * CHUNK : (c + 1) * CHUNK],
                        R_CONST,
                        g_ps[:],
                        op0=mult,
                        op1=mult,
                        accum_out=acc4[:, c : c + 1],
                    )

                # s = rowsum(acc4) ; v = F - s ; out = (x * -B) + v
                s_acc = accpool.tile([P, 1], f32, tag="s")
                nc.vector.tensor_reduce(s_acc, acc4[:], mybir.AxisListType.X, add)
                v = accpool.tile([P, 1], f32, tag="v")
                nc.scalar.activation(
                    v, s_acc, mybir.ActivationFunctionType.Copy,
                    bias=F_CONST, scale=-1.0,
                )
                o_sb = spool.tile([P, DIM], f32, tag="o")
                nc.vector.tensor_scalar(
                    out=o_sb, in0=xl_sb, scalar1=-B_CONST, scalar2=v,
                    op0=mult, op1=add,
                )
                nc.sync.dma_start(out=out[s * P : (s + 1) * P, :], in_=o_sb[:])

    nc.finalize()
    return nc


def _get_nc():
    if "nc" not in _CACHE:
        _CACHE["nc"] = _build_nc()
    return _CACHE["nc"]


def _make_in_maps(x, A):
    import ml_dtypes

    bf16 = ml_dtypes.bfloat16
    x = np.ascontiguousarray(np.asarray(x, dtype=np.float32))
    A = np.ascontiguousarray(np.asarray(A, dtype=np.float32))
    xt = np.ascontiguousarray(x.T)
    xt_hi = xt.astype(bf16)
    xt_lo = (xt - xt_hi.astype(np.float32)).astype(bf16)
    in_maps = []
    for c in range(NCORES):
        rows = slice(c * ROWS, (c + 1) * ROWS)
        in_maps.append(
            {
                "a": np.ascontiguousarray(A[rows]),
                "xt_hi": xt_hi,
                "xt_lo": xt_lo,
                "xlt_hi": np.ascontiguousarray(xt_hi[:, rows]),
                "xlt_lo": np.ascontiguousarray(xt_lo[:, rows]),
                "xloc": np.ascontiguousarray(x[rows]),
            }
        )
    return in_maps


def run_sharded(x, A, trace=False, **kwargs):
    """Run the SPMD bass kernel; returns (full_output, BassKernelResults)."""
    from concourse.bass_utils import run_bass_kernel_spmd

    nc = _get_nc()
    res = run_bass_kernel_spmd(
        nc, _make_in_maps(x, A), core_ids=list(range(NCORES)), trace=trace, **kwargs
    )
    full = np.concatenate([res.results[c]["out"] for c in range(NCORES)], axis=0)
    return full.astype(np.float32, copy=False), res


def kernel(t, x, A):
    out, _ = run_sharded(x, A)
    return out


# revision 10
# speedup vs baseline: 1.7465x; 1.3721x over previous
"""Trainium2 Bass kernel for BiochemicalDynamics.

Reference computation (f32):
    Ax    = A @ x                                   # [N, DIM]
    s     = R * rowsum(x * Ax)                      # [N, 1]
    out   = F - B*x - s                             # [N, DIM]

Key identity used on-device: the output only needs the per-row scalar
    s_i = R * sum_j A[i,j] * <x_i, x_j> = R * rowsum_j (A ⊙ G)[i,j]
with G = x @ x.T. G tiles are produced on the TensorEngine from xT
(stationary xT[:, rows_i], moving xT[:, cols_j]) — so A is consumed in
its natural row-major layout and never needs a transpose. A single
fused VectorEngine op (tensor_tensor_reduce) multiplies the A chunk by
the G chunk from PSUM and row-reduces it, chaining the per-partition
accumulator across chunks.

Sharding: row-shard A (and x rows) across the 8 cores; every core gets
the full xT (the "all-gather of x" is done host-side by replicating the
2MB input). No cross-core reduction is needed.
"""

import sys

import numpy as np

for _p in ("/opt/trn_rl_repo", "/root/.axon_site/_ro/trn_rl_repo"):
    if _p not in sys.path:
        sys.path.append(_p)

N = 8192
DIM = 64
NCORES = 8
ROWS = N // NCORES  # 1024 rows of A per core

F_CONST = 1.0
B_CONST = 0.1
R_CONST = 0.01

P = 128                  # SBUF partitions
NSTRIPES = ROWS // P     # 8 row-stripes per core
CHUNK = 2048             # columns per fused multiply-reduce (4 PSUM banks)
NCHUNKS = N // CHUNK     # 4
MM_N = 512               # matmul moving free dim (one PSUM bank, f32)
MM_PER_CHUNK = CHUNK // MM_N

_CACHE = {}


def _build_nc():
    import concourse.mybir as mybir
    import concourse.tile as tile
    from concourse import bacc

    f32 = mybir.dt.float32
    bf16 = mybir.dt.bfloat16

    nc = bacc.Bacc(
        trn_type="TRN2", target_bir_lowering=False, debug=False, num_devices=NCORES
    )

    a = nc.dram_tensor("a", [ROWS, N], f32, kind="ExternalInput")
    # x^T split into bf16 (hi, lo) pairs: x = hi + lo to ~2^-17. The G
    # matmuls run in bf16 (4-5x faster than fp32 on PE) with f32 PSUM
    # accumulation. K=128 packing: the stationary stacks [hi; lo] along
    # the contraction axis (DIM=64 each half) and the moving tensors
    # carry hi (resp. lo) duplicated in both halves, so two K=128
    # matmuls accumulate the exact product (hi+lo)·(hi+lo)^T:
    #   mm1 = hi_l·hi_r + lo_l·hi_r ;  mm2 = hi_l·lo_r + lo_l·lo_r
    xlt2 = nc.dram_tensor("xlt2", [2 * DIM, ROWS], bf16, kind="ExternalInput")
    xt2_hi = nc.dram_tensor("xt2_hi", [2 * DIM, N], bf16, kind="ExternalInput")
    xt2_lo = nc.dram_tensor("xt2_lo", [2 * DIM, N], bf16, kind="ExternalInput")
    xloc = nc.dram_tensor("xloc", [ROWS, DIM], f32, kind="ExternalInput")
    out = nc.dram_tensor("out", [ROWS, DIM], f32, kind="ExternalOutput")

    mult = mybir.AluOpType.mult
    add = mybir.AluOpType.add

    with tile.TileContext(nc) as tc:
        with (
            tc.tile_pool(name="xpool", bufs=1) as xpool,
            tc.tile_pool(name="apool", bufs=3) as apool,
            tc.tile_pool(name="spool", bufs=2) as spool,
            tc.tile_pool(name="accpool", bufs=2 * NCHUNKS) as accpool,
            tc.tile_pool(name="psum", bufs=2, space="PSUM") as psum_pool,
        ):
            # One-time loads: stacked x^T operands for the G matmuls.
            xt_hi_sb = xpool.tile([2 * DIM, N], bf16)
            nc.sync.dma_start(out=xt_hi_sb[:], in_=xt2_hi[:])
            xt_lo_sb = xpool.tile([2 * DIM, N], bf16)
            nc.sync.dma_start(out=xt_lo_sb[:], in_=xt2_lo[:])
            xlt_sb = xpool.tile([2 * DIM, ROWS], bf16)
            nc.sync.dma_start(out=xlt_sb[:], in_=xlt2[:])

            for s in range(NSTRIPES):
                a_sb = apool.tile([P, N], f32, tag="a")
                nc.sync.dma_start(out=a_sb[:], in_=a[s * P : (s + 1) * P, :])
                xl_sb = spool.tile([P, DIM], f32, tag="xl")
                nc.sync.dma_start(out=xl_sb[:], in_=xloc[s * P : (s + 1) * P, :])

                # acc4[:, c] = sum_j (A_chunk * R) * G_chunk  per chunk c,
                # via the fused DVE scalar_tensor_tensor accumulate output.
                acc4 = accpool.tile([P, NCHUNKS], f32, tag="acc4")
                lhsT = xlt_sb[:, s * P : (s + 1) * P]
                for c in range(NCHUNKS):
                    g_ps = psum_pool.tile([P, CHUNK], f32, tag="g")
                    for q in range(MM_PER_CHUNK):
                        col = c * CHUNK + q * MM_N
                        g_slice = g_ps[:, q * MM_N : (q + 1) * MM_N]
                        rt_hi = xt_hi_sb[:, col : col + MM_N]
                        rt_lo = xt_lo_sb[:, col : col + MM_N]
                        nc.tensor.matmul(g_slice, lhsT, rt_hi, start=True, stop=False)
                        nc.tensor.matmul(g_slice, lhsT, rt_lo, start=False, stop=True)
                    dummy = accpool.tile([P, 1], f32, tag="dummy")
                    nc.vector.scalar_tensor_tensor(
                        dummy.broadcast_to((P, CHUNK)),
                        a_sb[:, c * CHUNK : (c + 1) * CHUNK],
                        R_CONST,
                        g_ps[:],
                        op0=mult,
                        op1=mult,
                        accum_out=acc4[:, c : c + 1],
                    )

                # s = rowsum(acc4) ; v = F - s ; out = (x * -B) + v
                s_acc = accpool.tile([P, 1], f32, tag="s")
                nc.vector.tensor_reduce(s_acc, acc4[:], mybir.AxisListType.X, add)
                v = accpool.tile([P, 1], f32, tag="v")
                nc.scalar.activation(
                    v, s_acc, mybir.ActivationFunctionType.Copy,
                    bias=F_CONST, scale=-1.0,
                )
                o_sb = spool.tile([P, DIM], f32, tag="o")
                nc.vector.tensor_scalar(
                    out=o_sb, in0=xl_sb, scalar1=-B_CONST, scalar2=v,
                    op0=mult, op1=add,
                )
                nc.sync.dma_start(out=out[s * P : (s + 1) * P, :], in_=o_sb[:])

    nc.finalize()
    return nc


def _get_nc():
    if "nc" not in _CACHE:
        _CACHE["nc"] = _build_nc()
    return _CACHE["nc"]


def _make_in_maps(x, A):
    import ml_dtypes

    bf16 = ml_dtypes.bfloat16
    x = np.ascontiguousarray(np.asarray(x, dtype=np.float32))
    A = np.ascontiguousarray(np.asarray(A, dtype=np.float32))
    xt = np.ascontiguousarray(x.T)
    xt_hi = xt.astype(bf16)
    xt_lo = (xt - xt_hi.astype(np.float32)).astype(bf16)
    xt2_hi = np.ascontiguousarray(np.vstack([xt_hi, xt_hi]))
    xt2_lo = np.ascontiguousarray(np.vstack([xt_lo, xt_lo]))
    xlt2 = np.vstack([xt_hi, xt_lo])
    in_maps = []
    for c in range(NCORES):
        rows = slice(c * ROWS, (c + 1) * ROWS)
        in_maps.append(
            {
                "a": np.ascontiguousarray(A[rows]),
                "xt2_hi": xt2_hi,
                "xt2_lo": xt2_lo,
                "xlt2": np.ascontiguousarray(xlt2[:, rows]),
                "xloc": np.ascontiguousarray(x[rows]),
            }
        )
    return in_maps


def run_sharded(x, A, trace=False, **kwargs):
    """Run the SPMD bass kernel; returns (full_output, BassKernelResults)."""
    from concourse.bass_utils import run_bass_kernel_spmd

    nc = _get_nc()
    res = run_bass_kernel_spmd(
        nc, _make_in_maps(x, A), core_ids=list(range(NCORES)), trace=trace, **kwargs
    )
    full = np.concatenate([res.results[c]["out"] for c in range(NCORES)], axis=0)
    return full.astype(np.float32, copy=False), res


def kernel(t, x, A):
    out, _ = run_sharded(x, A)
    return out


# revision 15
# speedup vs baseline: 1.8857x; 1.0797x over previous
"""Trainium2 Bass kernel for BiochemicalDynamics.

Reference computation (f32):
    Ax    = A @ x                                   # [N, DIM]
    s     = R * rowsum(x * Ax)                      # [N, 1]
    out   = F - B*x - s                             # [N, DIM]

Key identity used on-device: the output only needs the per-row scalar
    s_i = R * sum_j A[i,j] * <x_i, x_j> = R * rowsum_j (A ⊙ G)[i,j]
with G = x @ x.T. G tiles are produced on the TensorEngine from xT
(stationary xT[:, rows_i], moving xT[:, cols_j]) — so A is consumed in
its natural row-major layout and never needs a transpose. A single
fused VectorEngine op (tensor_tensor_reduce) multiplies the A chunk by
the G chunk from PSUM and row-reduces it, chaining the per-partition
accumulator across chunks.

Sharding: row-shard A (and x rows) across the 8 cores; every core gets
the full xT (the "all-gather of x" is done host-side by replicating the
2MB input). No cross-core reduction is needed.
"""

import sys

import numpy as np

for _p in ("/opt/trn_rl_repo", "/root/.axon_site/_ro/trn_rl_repo"):
    if _p not in sys.path:
        sys.path.append(_p)

N = 8192
DIM = 64
NCORES = 8
ROWS = N // NCORES  # 1024 rows of A per core

F_CONST = 1.0
B_CONST = 0.1
R_CONST = 0.01

P = 128                  # SBUF partitions
NSTRIPES = ROWS // P     # 8 row-stripes per core
CHUNK = 2048             # columns per fused multiply-reduce (4 PSUM banks)
NCHUNKS = N // CHUNK     # 4
MM_N = 512               # matmul moving free dim (one PSUM bank, f32)
MM_PER_CHUNK = CHUNK // MM_N

_CACHE = {}


def _build_nc():
    import concourse.mybir as mybir
    import concourse.tile as tile
    from concourse import bacc

    f32 = mybir.dt.float32
    bf16 = mybir.dt.bfloat16

    nc = bacc.Bacc(
        trn_type="TRN2", target_bir_lowering=False, debug=False, num_devices=NCORES
    )

    a = nc.dram_tensor("a", [ROWS, N], f32, kind="ExternalInput")
    # x^T split into bf16 (hi, lo) pairs: x = hi + lo to ~2^-17. The G
    # matmuls run in bf16 (4-5x faster than fp32 on PE) with f32 PSUM
    # accumulation. K=128 packing: the stationary stacks [hi; lo] along
    # the contraction axis (DIM=64 each half) and the moving tensors
    # carry hi (resp. lo) duplicated in both halves, so two K=128
    # matmuls accumulate the exact product (hi+lo)·(hi+lo)^T:
    # The moving tensor is the stacked [hi; lo] x^T; the two stationaries
    # are [hi; lo] and the swapped [lo; hi], so the two K=128 matmuls give
    #   mm1 = hi_l·hi_r + lo_l·lo_r ;  mm2 = lo_l·hi_r + hi_l·lo_r
    # summing to exactly (hi+lo)·(hi+lo)^T in f32 PSUM.
    xlt_a = nc.dram_tensor("xlt_a", [2 * DIM, ROWS], bf16, kind="ExternalInput")
    xlt_b = nc.dram_tensor("xlt_b", [2 * DIM, ROWS], bf16, kind="ExternalInput")
    xt2 = nc.dram_tensor("xt2", [2 * DIM, N], bf16, kind="ExternalInput")
    xloc = nc.dram_tensor("xloc", [ROWS, DIM], f32, kind="ExternalInput")
    out = nc.dram_tensor("out", [ROWS, DIM], f32, kind="ExternalOutput")

    mult = mybir.AluOpType.mult
    add = mybir.AluOpType.add

    with tile.TileContext(nc) as tc:
        with (
            tc.tile_pool(name="xpool", bufs=1) as xpool,
            tc.tile_pool(name="apool", bufs=4) as apool,
            tc.tile_pool(name="spool", bufs=2) as spool,
            tc.tile_pool(name="accpool", bufs=2 * NCHUNKS) as accpool,
            tc.tile_pool(name="psum", bufs=2, space="PSUM") as psum_pool,
        ):
            # One-time loads: stacked x^T operands for the G matmuls.
            xt2_sb = xpool.tile([2 * DIM, N], bf16)
            nc.sync.dma_start(out=xt2_sb[:], in_=xt2[:])
            xlt_a_sb = xpool.tile([2 * DIM, ROWS], bf16)
            nc.sync.dma_start(out=xlt_a_sb[:], in_=xlt_a[:])
            xlt_b_sb = xpool.tile([2 * DIM, ROWS], bf16)
            nc.sync.dma_start(out=xlt_b_sb[:], in_=xlt_b[:])

            for s in range(NSTRIPES):
                a_sb = apool.tile([P, N], f32, tag="a")
                nc.sync.dma_start(out=a_sb[:], in_=a[s * P : (s + 1) * P, :])
                xl_sb = spool.tile([P, DIM], f32, tag="xl")
                nc.sync.dma_start(out=xl_sb[:], in_=xloc[s * P : (s + 1) * P, :])

                # acc4[:, c] = sum_j (A_chunk * R) * G_chunk  per chunk c,
                # via the fused DVE scalar_tensor_tensor accumulate output.
                acc4 = accpool.tile([P, NCHUNKS], f32, tag="acc4")
                lhsT_a = xlt_a_sb[:, s * P : (s + 1) * P]
                lhsT_b = xlt_b_sb[:, s * P : (s + 1) * P]
                for c in range(NCHUNKS):
                    g_ps = psum_pool.tile([P, CHUNK], f32, tag="g")
                    # All lhsT_a matmuls, then all lhsT_b: 2 weight loads
                    # per chunk instead of one per matmul.
                    for q in range(MM_PER_CHUNK):
                        col = c * CHUNK + q * MM_N
                        nc.tensor.matmul(
                            g_ps[:, q * MM_N : (q + 1) * MM_N],
                            lhsT_a, xt2_sb[:, col : col + MM_N],
                            start=True, stop=False,
                        )
                    for q in range(MM_PER_CHUNK):
                        col = c * CHUNK + q * MM_N
                        nc.tensor.matmul(
                            g_ps[:, q * MM_N : (q + 1) * MM_N],
                            lhsT_b, xt2_sb[:, col : col + MM_N],
                            start=False, stop=True,
                        )
                    dummy = accpool.tile([P, 1], f32, tag="dummy")
                    nc.vector.scalar_tensor_tensor(
                        dummy.broadcast_to((P, CHUNK)),
                        a_sb[:, c * CHUNK : (c + 1) * CHUNK],
                        R_CONST,
                        g_ps[:],
                        op0=mult,
                        op1=mult,
                        accum_out=acc4[:, c : c + 1],
                    )

                # s = rowsum(acc4) ; v = F - s ; out = (x * -B) + v
                s_acc = accpool.tile([P, 1], f32, tag="s")
                nc.vector.tensor_reduce(s_acc, acc4[:], mybir.AxisListType.X, add)
                v = accpool.tile([P, 1], f32, tag="v")
                nc.scalar.activation(
                    v, s_acc, mybir.ActivationFunctionType.Copy,
                    bias=F_CONST, scale=-1.0,
                )
                o_sb = spool.tile([P, DIM], f32, tag="o")
                nc.vector.tensor_scalar(
                    out=o_sb, in0=xl_sb, scalar1=-B_CONST, scalar2=v,
                    op0=mult, op1=add,
                )
                nc.sync.dma_start(out=out[s * P : (s + 1) * P, :], in_=o_sb[:])

    nc.finalize()
    return nc


def _get_nc():
    if "nc" not in _CACHE:
        _CACHE["nc"] = _build_nc()
    return _CACHE["nc"]


def _make_in_maps(x, A):
    import ml_dtypes

    bf16 = ml_dtypes.bfloat16
    x = np.ascontiguousarray(np.asarray(x, dtype=np.float32))
    A = np.ascontiguousarray(np.asarray(A, dtype=np.float32))
    xt = np.ascontiguousarray(x.T)
    xt_hi = xt.astype(bf16)
    xt_lo = (xt - xt_hi.astype(np.float32)).astype(bf16)
    xt2 = np.ascontiguousarray(np.vstack([xt_hi, xt_lo]))
    xlt_a = xt2
    xlt_b = np.ascontiguousarray(np.vstack([xt_lo, xt_hi]))
    in_maps = []
    for c in range(NCORES):
        rows = slice(c * ROWS, (c + 1) * ROWS)
        in_maps.append(
            {
                "a": np.ascontiguousarray(A[rows]),
                "xt2": xt2,
                "xlt_a": np.ascontiguousarray(xlt_a[:, rows]),
                "xlt_b": np.ascontiguousarray(xlt_b[:, rows]),
                "xloc": np.ascontiguousarray(x[rows]),
            }
        )
    return in_maps


def run_sharded(x, A, trace=False, **kwargs):
    """Run the SPMD bass kernel; returns (full_output, BassKernelResults)."""
    from concourse.bass_utils import run_bass_kernel_spmd

    nc = _get_nc()
    res = run_bass_kernel_spmd(
        nc, _make_in_maps(x, A), core_ids=list(range(NCORES)), trace=trace, **kwargs
    )
    full = np.concatenate([res.results[c]["out"] for c in range(NCORES)], axis=0)
    return full.astype(np.float32, copy=False), res


def kernel(t, x, A):
    out, _ = run_sharded(x, A)
    return out
